# revision 1
# baseline (speedup 1.0000x reference)
"""ChebNet (K=3, 3 layers) on trn2, node-sharded across 8 cores.

Math (per layer): out = h@(W0-W2) + P1@W1 + P2@(2*W2) + b,  P1 = L h, P2 = L P1,
L = -D^-1/2 A D^-1/2 (deg = out-degree over src).  relu after layers 0,1.

Device scheme per core (owns a contiguous slice of dst nodes):
 - features live transposed in SBUF: [128 feat (partitions), nodes (free)]
 - propagation: dma_gather rows of the (replicated, node-major) feature table
   for each edge (sorted by (block of 128 dst, bucket of <=32768 src rows)),
   128 edges -> 128 partitions; segmented sum via matmul:
     psum[f, d] += sum_e E_tile[e, f] * W2_tile[e, d]
   where W2_tile[e, d] = edge_weight if (dst_local % 128)==d else 0
   (host-precomputed, streamed from DRAM).
 - gather tables for the next prop are produced by PE-transposing each
   128-node block and AllGather-ing the node-major slices.
"""

from dataclasses import dataclass, field

import numpy as np
import ml_dtypes

import concourse.bass as bass
import concourse.bacc as bacc
import concourse.mybir as mybir
import concourse.tile as tile
from concourse import library_config
from concourse.tile import TileContext

BF16 = mybir.dt.bfloat16
F32 = mybir.dt.float32
I16 = mybir.dt.int16
AF = mybir.ActivationFunctionType
GMAXT = 8   # max 128-idx tiles per dma_gather call (ucode ring limit 1024)


@dataclass
class Meta:
    N: int
    C: int          # in/hidden feature dim (must be 128)
    COUT: int
    NCORES: int
    BWIN: int
    SCB: int
    W2CHUNK: int    # blocks per w2 dma chunk
    NPC: int = 0
    NBLK: int = 0
    NBUCK: int = 0
    T: object = None              # [NBLK, NBUCK] tiles per (blk, bucket)
    sc_blocks: list = field(default_factory=list)
    TOT_TILES: int = 0
    TOT_IDXCOLS: int = 0
    ts_sc: list = None            # [sc][b] tiles in gather call
    idx_seg_col: list = None      # [sc][b] column offset of call segment
    idx_sc_col: list = None       # [sc] col base of sc idx chunk
    idx_sc_cols: list = None      # [sc] col count of sc idx chunk
    eoff: list = None             # [sc][b][blk] tile offset in (sc,b) buffer
    mm_tile: dict = None          # (blk,b,t) -> global tile index (MM order)

    def finalize(self):
        assert self.C == 128
        self.sc_blocks = [
            list(range(s, min(s + self.SCB, self.NBLK)))
            for s in range(0, self.NBLK, self.SCB)
        ]
        nsc = len(self.sc_blocks)
        self.ts_sc = [[0] * self.NBUCK for _ in range(nsc)]
        self.eoff = [[dict() for _ in range(self.NBUCK)] for _ in range(nsc)]
        self.mm_tile = {}
        g = 0
        for si, blks in enumerate(self.sc_blocks):
            for b in range(self.NBUCK):
                off = 0
                for blk in blks:
                    self.eoff[si][b][blk] = off
                    off += int(self.T[blk, b])
                self.ts_sc[si][b] = off
            for blk in blks:
                for b in range(self.NBUCK):
                    for t in range(int(self.T[blk, b])):
                        self.mm_tile[(blk, b, t)] = g
                        g += 1
        self.TOT_TILES = g
        self.idx_seg_col = [[0] * self.NBUCK for _ in range(nsc)]
        self.idx_sc_col = [0] * nsc
        self.idx_sc_cols = [0] * nsc
        col = 0
        for si in range(nsc):
            self.idx_sc_col[si] = col
            for b in range(self.NBUCK):
                self.idx_seg_col[si][b] = col
                col += self.ts_sc[si][b] * 8
            self.idx_sc_cols[si] = col - self.idx_sc_col[si]
        self.TOT_IDXCOLS = col


def make_meta(N, C, COUT, ncores, edge_index, bwin=32768, scb=8, w2chunk=2):
    m = Meta(N=N, C=C, COUT=COUT, NCORES=ncores, BWIN=bwin, SCB=scb,
             W2CHUNK=w2chunk)
    m.NPC = N // ncores
    assert m.NPC * ncores == N
    m.NBLK = (m.NPC + 127) // 128
    m.NBUCK = (N + bwin - 1) // bwin
    assert m.NBUCK <= 4
    src = np.asarray(edge_index[0], dtype=np.int64)
    dst = np.asarray(edge_index[1], dtype=np.int64)
    core = dst // m.NPC
    blk = (dst - core * m.NPC) // 128
    buck = src // bwin
    lin = (core * m.NBLK + blk) * m.NBUCK + buck
    cnt = np.bincount(lin, minlength=ncores * m.NBLK * m.NBUCK).reshape(
        ncores, m.NBLK, m.NBUCK)
    m.T = np.ceil(cnt / 128.0).astype(np.int64).max(axis=0)
    m.finalize()
    return m


def prep_inputs(meta, x, edge_index, Ws, bs, table_dtype=ml_dtypes.bfloat16, w2_mode="dve"):
    """Returns per-core input dict list."""
    m = meta
    N, C = m.N, m.C
    src = np.asarray(edge_index[0], dtype=np.int64)
    dst = np.asarray(edge_index[1], dtype=np.int64)
    deg = np.bincount(src, minlength=N).astype(np.float64)
    dinv = np.where(deg > 0, 1.0 / np.sqrt(np.maximum(deg, 1e-30)), 0.0)
    w = (-(dinv[src] * dinv[dst])).astype(np.float32)

    x = np.asarray(x, dtype=np.float32)
    shared = {}
    shared["x_table"] = np.ascontiguousarray(x.astype(table_dtype))
    shared["ident"] = np.eye(128, dtype=table_dtype)
    shared["iotat"] = np.ascontiguousarray(
        np.tile(np.arange(128, dtype=np.float32), (128, 1)))
    for l in range(3):
        W = np.asarray(Ws[l], dtype=np.float32)
        shared[f"wA{l}"] = np.ascontiguousarray((W[0] - W[2]).astype(table_dtype))
        shared[f"wB{l}"] = np.ascontiguousarray(W[1].astype(table_dtype))
        shared[f"wC{l}"] = np.ascontiguousarray((2.0 * W[2]).astype(table_dtype))
        bias = np.zeros((128, 1), dtype=np.float32)
        bias[: bs[l].shape[0], 0] = np.asarray(bs[l], dtype=np.float32)
        shared[f"bias{l}"] = bias

    core = dst // m.NPC
    per_core = []
    for c in range(m.NCORES):
        sel = np.nonzero(core == c)[0]
        s_c = src[sel]
        d_c = dst[sel] - c * m.NPC
        w_c = w[sel]
        blk_c = d_c // 128
        buck_c = s_c // m.BWIN
        dcol_c = d_c % 128
        lidx_c = s_c - buck_c * m.BWIN

        order = np.lexsort((buck_c, blk_c))
        blk_s, buck_s = blk_c[order], buck_c[order]
        lidx_s, w_s, dcol_s = lidx_c[order], w_c[order], dcol_c[order]
        grp = blk_s * m.NBUCK + buck_s
        starts = np.searchsorted(grp, np.arange(m.NBLK * m.NBUCK), side="left")
        ends = np.searchsorted(grp, np.arange(m.NBLK * m.NBUCK), side="right")
        ent = {}
        for blk in range(m.NBLK):
            for b in range(m.NBUCK):
                t = int(m.T[blk, b])
                if t == 0:
                    continue
                gid = blk * m.NBUCK + b
                s0, s1 = int(starts[gid]), int(ends[gid])
                L = t * 128
                n = s1 - s0
                assert n <= L, (n, L, blk, b)
                ei = np.zeros(L, dtype=np.int16)
                ew = np.zeros(L, dtype=np.float32)
                ed = np.zeros(L, dtype=np.int64)
                ei[:n] = lidx_s[s0:s1].astype(np.int16)
                ew[:n] = w_s[s0:s1]
                ed[:n] = dcol_s[s0:s1]
                ent[(blk, b)] = (ei, ew, ed)

        # w2 stream [128, TOT_TILES*128] in MM order (stream mode) or
        # per-tile (off, w) scalars [128, 2*TOT_TILES] (dve mode)
        if w2_mode == "stream":
            w2 = np.zeros((128, m.TOT_TILES * 128), dtype=np.float32)
            p128 = np.arange(128)
            for (blk, b), (ei, ew, ed) in ent.items():
                for ti in range(int(m.T[blk, b])):
                    g = m.mm_tile[(blk, b, ti)]
                    w2[p128, g * 128 + ed[ti * 128:(ti + 1) * 128]] = \
                        ew[ti * 128:(ti + 1) * 128]
        else:
            w2 = np.zeros((128, m.TOT_TILES * 2), dtype=np.float32)
            for (blk, b), (ei, ew, ed) in ent.items():
                for ti in range(int(m.T[blk, b])):
                    g = m.mm_tile[(blk, b, ti)]
                    w2[:, 2 * g] = ed[ti * 128:(ti + 1) * 128]
                    w2[:, 2 * g + 1] = ew[ti * 128:(ti + 1) * 128]
        # idx stream [128, TOT_IDXCOLS] in gather-call order
        idxs = np.zeros((128, max(1, m.TOT_IDXCOLS)), dtype=np.int16)
        for si, blks in enumerate(m.sc_blocks):
            for b in range(m.NBUCK):
                seg = [ent[(blk, b)][0] for blk in blks if (blk, b) in ent]
                if not seg:
                    continue
                seg = np.concatenate(seg)
                cols = seg.reshape(-1, 16).T  # [16, L/16]
                c0 = m.idx_seg_col[si][b]
                idxs[:, c0: c0 + cols.shape[1]] = np.tile(cols, (8, 1))

        xT = np.zeros((128, m.NBLK * 128), dtype=table_dtype)
        xT[:C, : m.NPC] = x[c * m.NPC:(c + 1) * m.NPC, :].T.astype(table_dtype)

        d = dict(shared)
        d["w2"] = np.ascontiguousarray(
            w2.astype(table_dtype if w2_mode == "stream" else np.float32))
        d["idxs"] = idxs
        d["xT"] = xT
        per_core.append(d)
    return per_core


def build_nc(meta, table_mybir_dt=BF16, repeat=1, skip=(), ncores_override=None, gmaxt=None, nqueues=4, ebuf_bufs=3, w2_mode="dve"):
    m = meta
    ncores = ncores_override or m.NCORES
    gmaxt = gmaxt or GMAXT
    TD = table_mybir_dt
    nc = bacc.Bacc("TRN2", target_bir_lowering=False, debug=False,
                   num_devices=ncores, num_swdge_queues=4)

    x_table = nc.dram_tensor("x_table", [m.N, m.C], TD, kind="ExternalInput")
    xT_in = nc.dram_tensor("xT", [128, m.NBLK * 128], TD, kind="ExternalInput")
    idx_in = nc.dram_tensor("idxs", [128, max(1, m.TOT_IDXCOLS)], I16,
                            kind="ExternalInput")
    if w2_mode == "stream":
        w2_in = nc.dram_tensor("w2", [128, m.TOT_TILES * 128], TD,
                               kind="ExternalInput")
    else:
        w2_in = nc.dram_tensor("w2", [128, m.TOT_TILES * 2], F32,
                               kind="ExternalInput")
    iota_in = nc.dram_tensor("iotat", [128, 128], F32, kind="ExternalInput")
    ident_in = nc.dram_tensor("ident", [128, 128], TD, kind="ExternalInput")
    wd_in, bias_in = {}, {}
    for l in range(3):
        co = m.COUT if l == 2 else m.C
        for nm in ("A", "B", "C"):
            wd_in[(l, nm)] = nc.dram_tensor(f"w{nm}{l}", [128, co], TD,
                                            kind="ExternalInput")
        bias_in[l] = nc.dram_tensor(f"bias{l}", [128, 1], F32,
                                    kind="ExternalInput")
    out_dram = nc.dram_tensor("outT", [m.COUT, m.NBLK * 128], F32,
                              kind="ExternalOutput")

    groups = [list(range(ncores))]
    _qc = [0]

    def next_q():
        _qc[0] = (_qc[0] + 1) % nqueues
        return _qc[0]

    with TileContext(nc) as tc:
        with (
            tc.tile_pool(name="const", bufs=1) as constp,
            tc.tile_pool(name="feat", bufs=1) as featp,
            tc.tile_pool(name="idxp", bufs=3) as idxp,
            tc.tile_pool(name="w2p", bufs=3) as w2p,
            tc.tile_pool(name="e0", bufs=ebuf_bufs) as ep0,
            tc.tile_pool(name="e1", bufs=ebuf_bufs) as ep1,
            tc.tile_pool(name="e2", bufs=ebuf_bufs) as ep2,
            tc.tile_pool(name="e3", bufs=ebuf_bufs) as ep3,
            tc.tile_pool(name="stage", bufs=6) as stagep,
            tc.tile_pool(name="w2t", bufs=12) as w2tp,
            tc.tile_pool(name="acc", bufs=4, space="PSUM") as accp,
            tc.tile_pool(name="tp", bufs=2, space="PSUM") as tpp,
            tc.tile_pool(name="dn", bufs=2, space="PSUM") as dnp,
            tc.tile_pool(name="dram", bufs=1, space="DRAM") as dramp,
        ):
            epools = [ep0, ep1, ep2, ep3]

            ident = constp.tile([128, 128], TD)
            nc.sync.dma_start(ident[:], ident_in[:, :])
            iota_sb = constp.tile([128, 128], F32)
            nc.sync.dma_start(iota_sb[:], iota_in[:, :])
            wd_sb, bias_sb = {}, {}
            for l in range(3):
                co = m.COUT if l == 2 else m.C
                for nm in ("A", "B", "C"):
                    t = constp.tile([128, co], TD, tag=f"w{nm}{l}")
                    nc.sync.dma_start(t[:], wd_in[(l, nm)][:, :])
                    wd_sb[(l, nm)] = t
                bt = constp.tile([128, 1], F32, tag=f"bias{l}")
                nc.sync.dma_start(bt[:], bias_in[l][:, :])
                bias_sb[l] = bt

            featA = featp.tile([128, m.NBLK * 128], TD, tag="featA")
            nc.sync.dma_start(featA[:], xT_in[:, :])
            featB = featp.tile([128, m.NBLK * 128], TD, tag="featB")
            p1T = featp.tile([128, m.NBLK * 128], TD, tag="p1T")

            lib_inst = nc.gpsimd.load_library(library_config.mlp)
            lib_pin = lib_inst.ins

            tbl_p1 = [[dramp.tile([m.N, m.C], TD, name=f"tblp1_{l}_r{r}",
                                  addr_space="Shared", tag=f"tblp1_{l}_r{r}")
                       for l in range(3)] for r in range(repeat)]
            ag_p1 = [dramp.tile([m.NPC, m.C], TD, name=f"agp1_{l}",
                                tag=f"agp1_{l}") for l in range(3)]
            tbl_h = [[dramp.tile([m.N, m.C], TD, name=f"tblh_{l}_r{r}",
                                 addr_space="Shared", tag=f"tblh_{l}_r{r}")
                      for l in range(2)] for r in range(repeat)]
            ag_h = [dramp.tile([m.NPC, m.C], TD, name=f"agh_{l}",
                               tag=f"agh_{l}") for l in range(2)]

            def bucket_rows(tbl_ap):
                out = []
                for b in range(m.NBUCK):
                    r0 = b * m.BWIN
                    r1 = min((b + 1) * m.BWIN, m.N)
                    out.append(tbl_ap[r0:r1, :])
                return out

            def emit_table_block(feat_sb, blk, ag_tile):
                tp = tpp.tile([128, 128], TD, tag="tp")
                nc.tensor.transpose(
                    tp[:], feat_sb[:, blk * 128:(blk + 1) * 128], ident[:])
                stg = stagep.tile([128, 128], TD, tag="tstage")
                nc.vector.tensor_copy(stg[:], tp[:])
                nvalid = min(128, m.NPC - blk * 128)
                nc.sync.dma_start(ag_tile[blk * 128: blk * 128 + nvalid, :],
                                  stg[:nvalid, :])

            def dense_block(l, feat_in, p2_psum, blk):
                co = m.COUT if l == 2 else m.C
                cols = slice(blk * 128, (blk + 1) * 128)
                p2s = stagep.tile([128, 128], TD, tag="p2stage")
                nc.vector.tensor_copy(p2s[:], p2_psum[:])
                dn = dnp.tile([128, 128], F32, tag="dn")
                nc.tensor.matmul(dn[:co, :], wd_sb[(l, "A")][:, :],
                                 feat_in[:, cols], start=True, stop=False)
                nc.tensor.matmul(dn[:co, :], wd_sb[(l, "B")][:, :],
                                 p1T[:, cols], start=False, stop=False)
                nc.tensor.matmul(dn[:co, :], wd_sb[(l, "C")][:, :],
                                 p2s[:], start=False, stop=True)
                if l < 2:
                    outf = featB if l == 0 else featA
                    nc.scalar.activation(outf[:, cols], dn[:, :], AF.Relu,
                                         bias=bias_sb[l][:, :])
                    emit_table_block(outf, blk, ag_h[l])
                else:
                    stg = stagep.tile([m.COUT, 128], F32, tag="ostage")
                    nc.scalar.activation(stg[:], dn[:co, :], AF.Identity,
                                         bias=bias_sb[l][:co, :])
                    nc.sync.dma_start(out_dram[:, cols], stg[:])

            def prop(tbl_aps, out_feat=None, make_tbl_ag=None, dense=None):
                for si, blks in enumerate(m.sc_blocks):
                    idx_sb = idxp.tile([128, max(8, max(m.idx_sc_cols))], I16,
                                       tag="idx")
                    if m.idx_sc_cols[si]:
                        nc.sync.dma_start(
                            idx_sb[:, : m.idx_sc_cols[si]],
                            idx_in[:, m.idx_sc_col[si]:
                                   m.idx_sc_col[si] + m.idx_sc_cols[si]])
                    ebufs = {}
                    for b in range(m.NBUCK):
                        ts = m.ts_sc[si][b]
                        if ts == 0:
                            continue
                        ebuf = epools[b].tile([128, ts, 128], TD, tag=f"e{b}")
                        c0 = m.idx_seg_col[si][b] - m.idx_sc_col[si]
                        if "gather" in skip:
                            nc.vector.memset(ebuf[:, 0:1, :], 0.0)
                        for t0 in range(0, ts, gmaxt) if "gather" not in skip else ():
                            tk = min(gmaxt, ts - t0)
                            g = nc.gpsimd.dma_gather(
                                ebuf[:, t0: t0 + tk, :], tbl_aps[b],
                                idx_sb[:, c0 + t0 * 8: c0 + (t0 + tk) * 8],
                                tk * 128, tk * 128, m.C,
                                queue_num=next_q())
                            tile.add_dep_helper(lib_pin, g.ins, sync=False,
                                                reason="lib before gather")
                        ebufs[b] = ebuf
                    for ci in range(0, len(blks), m.W2CHUNK):
                        cblks = blks[ci: ci + m.W2CHUNK]
                        ntile = sum(int(m.T[blk, b]) for blk in cblks
                                    for b in range(m.NBUCK))
                        if ntile == 0:
                            for blk in cblks:
                                _zero_block(nc, accp, out_feat, make_tbl_ag,
                                            dense, blk, emit_table_block,
                                            dense_block)
                            continue
                        g0 = min(m.mm_tile[(blk, b, 0)] for blk in cblks
                                 for b in range(m.NBUCK) if m.T[blk, b] > 0)
                        if w2_mode == "stream":
                            w2_sb = w2p.tile([128, ntile * 128], TD, tag="w2")
                            nc.sync.dma_start(
                                w2_sb[:], w2_in[:, g0 * 128:(g0 + ntile) * 128])
                        else:
                            w2_sb = w2p.tile([128, ntile * 2], F32, tag="w2")
                            nc.sync.dma_start(
                                w2_sb[:], w2_in[:, g0 * 2:(g0 + ntile) * 2])
                        for blk in cblks:
                            n_mm = sum(int(m.T[blk, b])
                                       for b in range(m.NBUCK))
                            acc = accp.tile([128, 128], F32, tag="acc")
                            i = 0
                            if "mm" in skip:
                                nc.vector.memset(acc[:], 0.0)
                                n_mm = 0
                            for b in (range(m.NBUCK) if "mm" not in skip else ()):
                                for t in range(int(m.T[blk, b])):
                                    gt = m.mm_tile[(blk, b, t)] - g0
                                    et = m.eoff[si][b][blk] + t
                                    if w2_mode == "stream":
                                        rhs = w2_sb[:, gt * 128:(gt + 1) * 128]
                                    else:
                                        w2t = w2tp.tile([128, 128], TD,
                                                        tag="w2t")
                                        nc.vector.tensor_scalar(
                                            w2t[:], iota_sb[:],
                                            w2_sb[:, 2 * gt: 2 * gt + 1],
                                            w2_sb[:, 2 * gt + 1: 2 * gt + 2],
                                            op0=mybir.AluOpType.is_equal,
                                            op1=mybir.AluOpType.mult)
                                        rhs = w2t[:]
                                    nc.tensor.matmul(
                                        acc[:],
                                        ebufs[b][:, et, :],
                                        rhs,
                                        start=(i == 0), stop=(i == n_mm - 1))
                                    i += 1
                            if n_mm == 0:
                                nc.vector.memset(acc[:], 0.0)
                            if out_feat is not None:
                                nc.vector.tensor_copy(
                                    out_feat[:, blk * 128:(blk + 1) * 128],
                                    acc[:])
                            if make_tbl_ag is not None:
                                assert out_feat is not None
                                emit_table_block(out_feat, blk, make_tbl_ag)
                            if dense is not None:
                                dense_block(dense[0], dense[1], acc, blk)

            def allgather(ag_tile, tbl_tile):
                if "ag" in skip:
                    return
                nc.gpsimd.collective_compute(
                    "AllGather", mybir.AluOpType.bypass,
                    replica_groups=groups,
                    ins=[ag_tile[:, :].opt()], outs=[tbl_tile[:, :].opt()])

            for rep in range(repeat):
                if rep > 0:
                    nc.sync.dma_start(featA[:], xT_in[:, :])
                for l in range(3):
                    feat_in = featA if l != 1 else featB
                    tbl_in = x_table if l == 0 else tbl_h[rep][l - 1]
                    prop(bucket_rows(tbl_in), out_feat=p1T,
                         make_tbl_ag=ag_p1[l])
                    allgather(ag_p1[l], tbl_p1[rep][l])
                    prop(bucket_rows(tbl_p1[rep][l]), dense=(l, feat_in))
                    if l < 2:
                        allgather(ag_h[l], tbl_h[rep][l])

    nc.compile()
    return nc


def _zero_block(nc, accp, out_feat, make_tbl_ag, dense, blk,
                emit_table_block, dense_block):
    acc = accp.tile([128, 128], F32, tag="acc")
    nc.vector.memset(acc[:], 0.0)
    if out_feat is not None:
        nc.vector.tensor_copy(out_feat[:, blk * 128:(blk + 1) * 128], acc[:])
    if make_tbl_ag is not None:
        emit_table_block(out_feat, blk, make_tbl_ag)
    if dense is not None:
        dense_block(dense[0], dense[1], acc, blk)


def assemble_output(meta, results):
    m = meta
    out = np.zeros((m.N, m.COUT), dtype=np.float32)
    for c in range(m.NCORES):
        o = results[c]["outT"]
        out[c * m.NPC:(c + 1) * m.NPC, :] = o[:, : m.NPC].T
    return out


def numpy_reference(x, edge_index, Ws, bs):
    src = np.asarray(edge_index[0], dtype=np.int64)
    dst = np.asarray(edge_index[1], dtype=np.int64)
    n = x.shape[0]
    deg = np.bincount(src, minlength=n).astype(np.float64)
    dinv = np.where(deg > 0, 1.0 / np.sqrt(np.maximum(deg, 1e-30)), 0.0)
    w = (-(dinv[src] * dinv[dst])).astype(np.float64)

    def prop(h):
        out = np.zeros_like(h)
        np.add.at(out, dst, w[:, None] * h[src])
        return out

    def cheb(h, W, b):
        Tx0, Tx1 = h, prop(h)
        out = Tx0 @ W[0] + Tx1 @ W[1]
        Tx2 = 2.0 * prop(Tx1) - Tx0
        out = out + Tx2 @ W[2]
        return out + b

    h = np.asarray(x, dtype=np.float64)
    h = np.maximum(cheb(h, Ws[0], bs[0]), 0.0)
    h = np.maximum(cheb(h, Ws[1], bs[1]), 0.0)
    return cheb(h, Ws[2], bs[2]).astype(np.float32)


# ---------------------------------------------------------------------------
# self-contained kernel entry point (full inputs in, full output out)
# ---------------------------------------------------------------------------

LAST_EXEC_NS = None
LAST_RESULTS = None


def kernel(**inputs):
    global LAST_EXEC_NS, LAST_RESULTS
    import numpy as _np
    from concourse.bass_utils import run_bass_kernel_spmd

    x = _np.asarray(inputs["x"], _np.float32)
    edge_index = _np.asarray(inputs["edge_index"], _np.int64)
    Ws = [_np.asarray(inputs[f"W{l}"], _np.float32) for l in range(3)]
    bs = [_np.asarray(inputs[f"b{l}"], _np.float32) for l in range(3)]

    meta = make_meta(100000, 128, 64, 8, edge_index,
                     bwin=32768, scb=8, w2chunk=2)
    per_core = prep_inputs(meta, x, edge_index, Ws, bs)
    nc = build_nc(meta)
    import os
    trace = os.environ.get("GNN_TRACE", "0") == "1"
    try:
        res = run_bass_kernel_spmd(nc, per_core, list(range(meta.NCORES)),
                                   trace=trace)
    except Exception:
        if not trace:
            raise
        res = run_bass_kernel_spmd(nc, per_core, list(range(meta.NCORES)),
                                   trace=False)
    LAST_EXEC_NS = res.exec_time_ns
    LAST_RESULTS = res
    return assemble_output(meta, res.results)

